# revision 10
# baseline (speedup 1.0000x reference)
"""Trainium2 Bass kernel for GCFM sparse-attention module.

Sharding: 8 cores = 2 batches x 4 row-blocks (12 rows each).
Each core gets reflect-padded input slabs (18 rows x 54 cols) so the
7x7 local-attention window needs no boundary logic on device; the
conv branch's zero-padding is handled with a 0/1 validity mask.

v4 restructure:
  - fine-grained tiles (per (kt,chk) x/y pieces, per-kt z, per-mt
    q/k/v/kms, per-pb vts, per-ig fs) so dependency tracking never
    over-serializes on whole-tensor granularity.
  - input DMAs split into ~2us pieces spread over the 3 DMA queues
    (sync/scalar/gpsimd) ordered by first-use time.
  - 14 warm-up matmuls on scratch data ramp the PE clock to max
    p-state while the first input DMAs are still in flight.
  - position encoding built on device from a rank-2 blob (aX + bY),
    conv validity mask shipped as one row and partition-broadcast.
  - kms = (k_psum + b2) - pe fused into one DVE op reading PSUM.
  - conv f-stage reads raw q/k/v slabs; validity mask applied at the
    psum->sbuf copy (mask commutes with channel mixing).
  - depthwise weights shipped compact ([117, 54, 52], 0.66MB instead
    of 1.7MB); each (tt,hf,ig) is a clean 9-matmul accumulation group
    in its own psum, copied to a staging tile and DMA'd to its
    channel range in DRAM.
  - vts carries no b3 bias (host adds 0.5*b3 after normalize).
  - att + conv outputs shipped as f16.

Math notes:
  - att logits = (1/8) * q . unfold(k - pe); the q.pe term is constant
    over the window axis and cancels in softmax.
  - AV rhs includes a constant 2.0 column so psum row 64 = 2*sum(exp);
    host computes att/row64 = 0.5 * softmax-weighted v + 0.5*b3.
  - QK runs chunk-major: one matmul per (head, 2-window-row chunk)
    covering all row-blocks that use the chunk (N up to 384).
"""

import os
import numpy as np

# ---- hardcoded problem geometry ----
B, C, H, W = 2, 256, 48, 48
HEAD, D = 4, 64
KATT = 7
PAD = KATT // 2          # 3
NCORES = 8
RPC = H // 4             # 12 rows per core
SLAB_R, SLAB_C = RPC + 2 * PAD, W + 2 * PAD   # 18, 54
PX = SLAB_R * SLAB_C     # 972
PXP = PX + 32            # padded slab stride so 128-wide chunk reads fit
CGR, CGC = RPC + 2, W + 2   # conv grid 14 x 50
CPX = CGR * CGC          # 700
GSIZE6 = [13, 13, 6, 13, 13, 6]
GSTART6 = [0, 13, 26, 32, 45, 58]
GM6 = [9 * g for g in GSIZE6]            # f-block col widths (117/54)
OFF_F = [0]
for _g in GM6:
    OFF_F.append(OFF_F[-1] + 6 * _g)     # compact cwf offsets, total 3456
NCWF = OFF_F[-1]
NCWD = 54 * 52                           # compact cwd cols

_CACHE = {}
last_results = None      # BassKernelResults from the most recent run


def _build_nc():
    """Build the (shared SPMD) Bass program once."""
    import concourse.bacc as bacc
    import concourse.mybir as mybir
    from concourse.tile import TileContext
    from contextlib import ExitStack

    F32 = mybir.dt.float32
    F16 = mybir.dt.float16
    AF = mybir.ActivationFunctionType
    ALU = mybir.AluOpType

    nc = bacc.Bacc(None, target_bir_lowering=False)

    w1_d = nc.declare_dram_parameter('w1', [128, 512], F16, isOutput=False)
    w2_d = nc.declare_dram_parameter('w2', [128, 512], F16, isOutput=False)
    w3_d = nc.declare_dram_parameter('w3', [128, 512], F16, isOutput=False)
    b32_d = nc.declare_dram_parameter('b32', [128, 6], F32, isOutput=False)
    sm_d = nc.declare_dram_parameter('p_sm', [128, 72], F32, isOutput=False)
    mar_d = nc.declare_dram_parameter('p_mar', [108, 384], F16, isOutput=False)
    mc_d = nc.declare_dram_parameter('p_mc', [128, CPX], F16, isOutput=False)
    px_d = nc.declare_dram_parameter('p_x', [128, 2 * PXP], F16, isOutput=False)
    py_d = nc.declare_dram_parameter('p_y', [128, 2 * PXP], F16, isOutput=False)
    pz_d = nc.declare_dram_parameter('p_z', [128, 2 * PXP], F16, isOutput=False)
    pcwf_d = nc.declare_dram_parameter('p_cwf', [128, NCWF], F16, isOutput=False)
    pcwd_d = nc.declare_dram_parameter('p_cwd', [117, NCWD], F16, isOutput=False)
    oatt_d = nc.declare_dram_parameter('out_att', [2, 65, 1152], F16, isOutput=True)
    ocv_d = nc.declare_dram_parameter('out_conv', [2, 2, 128, 288], F16, isOutput=True)

    with TileContext(nc) as tc, ExitStack() as ctx:
        P = ctx.enter_context(tc.tile_pool(name='persist', bufs=1))
        AE = ctx.enter_context(tc.tile_pool(name='attE', bufs=18))
        SG = ctx.enter_context(tc.tile_pool(name='stg', bufs=3))
        PS = ctx.enter_context(tc.tile_pool(name='ps', bufs=2, space='PSUM'))
        PSQ = ctx.enter_context(tc.tile_pool(name='psq', bufs=2, space='PSUM'))
        PSA = ctx.enter_context(tc.tile_pool(name='psa', bufs=4, space='PSUM'))

        # ---- persistent SBUF tensors ----
        w1t = P.tile([128, 512], F16, tag='w1t')
        w2t = P.tile([128, 512], F16, tag='w2t')
        w3t = P.tile([128, 512], F16, tag='w3t')
        b32 = P.tile([128, 6], F32, tag='b32')
        smt = P.tile([128, 72], F32, tag='smt')
        mart = P.tile([108, 384], F16, tag='mart')
        mct = P.tile([128, CPX], F16, tag='mct')
        xts = [P.tile([128, 9, SLAB_C], F16, tag=f'xt{i}', name=f'xts{i}') for i in range(4)]
        yts = [P.tile([128, 9, SLAB_C], F16, tag=f'yt{i}', name=f'yts{i}') for i in range(4)]
        ztc = [P.tile([128, PXP], F16, tag=f'zt{i}', name=f'ztc{i}') for i in range(2)]
        pes = P.tile([128, SLAB_R, SLAB_C], F16, tag='pes')
        cwft = P.tile([128, NCWF], F16, tag='cwft')
        cwdt = P.tile([117, NCWD], F16, tag='cwdt')

        qst = [P.tile([128, SLAB_R, SLAB_C], F16, tag=f'qs{i}', name=f'qst{i}') for i in range(2)]
        kst = [P.tile([128, SLAB_R, SLAB_C], F16, tag=f'ks{i}', name=f'kst{i}') for i in range(2)]
        vst = [P.tile([128, SLAB_R, SLAB_C], F16, tag=f'vs{i}', name=f'vst{i}') for i in range(2)]
        kmst = [P.tile([128, PXP], F16, tag=f'kms{i}', name=f'kmst{i}') for i in range(2)]
        vtsc = [P.tile([108, 4, 65], F16, tag=f'vts{i}', name=f'vtsc{i}') for i in range(9)]
        att0 = P.tile([65, 2, 6, 96], F16, tag='att0')
        att1 = P.tile([65, 2, 6, 96], F16, tag='att1')
        fsg = [P.tile([128, CGR, CGC], F16, tag=f'fs{i}', name=f'fsg{i}') for i in range(6)]
        wu = P.tile([128, 512], F16, tag='wu')

        # ---- input DMAs: 3 queues, ~2us pieces, ordered by first use ----
        def xy_in(dram, kt, chk):
            return dram[:, PXP * kt + 486 * chk:PXP * kt + 486 * chk + 486] \
                .rearrange('p (r c) -> p r c', c=SLAB_C)

        CWF3 = [0, 1404, 2430, NCWF]     # ig01 | ig23 | ig45
        nc.sync.dma_start(out=w1t[:], in_=w1_d[:])
        nc.sync.dma_start(out=xts[0][:], in_=xy_in(px_d, 0, 0))
        nc.sync.dma_start(out=w2t[:], in_=w2_d[:])
        nc.sync.dma_start(out=ztc[0][:, 0:486], in_=pz_d[:, 0:486])
        nc.sync.dma_start(out=w3t[:], in_=w3_d[:])
        nc.sync.dma_start(out=cwft[:, CWF3[0]:CWF3[1]], in_=pcwf_d[:, CWF3[0]:CWF3[1]])
        nc.sync.dma_start(out=cwdt[:, 0:936], in_=pcwd_d[:, 0:936])

        nc.scalar.dma_start(out=b32[:], in_=b32_d[:])
        nc.scalar.dma_start(out=smt[:], in_=sm_d[:])
        nc.scalar.dma_start(out=xts[2][:], in_=xy_in(px_d, 1, 0))
        nc.scalar.dma_start(out=yts[0][:], in_=xy_in(py_d, 0, 0))
        nc.scalar.dma_start(out=yts[2][:], in_=xy_in(py_d, 1, 0))
        nc.scalar.dma_start(out=ztc[0][:, 486:PXP], in_=pz_d[:, 486:PXP])
        nc.scalar.dma_start(out=mart[:], in_=mar_d[:])
        nc.scalar.dma_start(out=cwft[:, CWF3[1]:CWF3[2]], in_=pcwf_d[:, CWF3[1]:CWF3[2]])
        nc.scalar.dma_start(out=cwdt[:, 936:1872], in_=pcwd_d[:, 936:1872])

        nc.gpsimd.dma_start(out=xts[1][:], in_=xy_in(px_d, 0, 1))
        nc.gpsimd.dma_start(out=xts[3][:], in_=xy_in(px_d, 1, 1))
        nc.gpsimd.dma_start(out=yts[1][:], in_=xy_in(py_d, 0, 1))
        nc.gpsimd.dma_start(out=yts[3][:], in_=xy_in(py_d, 1, 1))
        nc.gpsimd.dma_start(out=ztc[1][:, 0:486], in_=pz_d[:, PXP:PXP + 486])
        nc.gpsimd.dma_start(out=ztc[1][:, 486:PXP], in_=pz_d[:, PXP + 486:2 * PXP])
        nc.gpsimd.dma_start(out=mct[:], in_=mc_d[:])
        nc.gpsimd.dma_start(out=cwft[:, CWF3[2]:CWF3[3]], in_=pcwf_d[:, CWF3[2]:CWF3[3]])
        nc.gpsimd.dma_start(out=cwdt[:, 1872:NCWD], in_=pcwd_d[:, 1872:NCWD])

        # ---- PE warm-up: ramp the clock while input DMAs fly ----
        nc.vector.memset(wu[:], 0.001)
        pwu = PS.tile([128, 512], F32, tag='ps', name='warm')
        for i in range(9):
            nc.tensor.matmul(pwu[:], lhsT=wu[:, 0:128], rhs=wu[:],
                             start=(i == 0), stop=(i == 8))
        nc.vector.tensor_scalar_add(wu[:, 0:2], pwu[:, 0:2], 0.0)

        # ---- small device-side constructions ----
        # pe[p, r, c] = aX[p, c] + bY[p, r]  (rank-2 position encoding)
        for r in range(SLAB_R):
            nc.vector.tensor_scalar_add(pes[:, r, :], smt[:, 0:SLAB_C],
                                        smt[:, SLAB_C + r:SLAB_C + r + 1])
        for mt in range(2):
            nc.vector.memset(kmst[mt][:, PX:PXP], 0.0)

        # ---- stage 1: 1x1 convs q,k,v ----
        def z_rhs(kt, chk):
            return ztc[kt][:, 0:PX].rearrange('p (r c) -> p r c', c=SLAB_C)[
                :, 9 * chk:9 * chk + 9, :]

        srcs1 = (
            (lambda kt, chk: xts[2 * kt + chk][:], w1t, 0, 'q'),
            (lambda kt, chk: yts[2 * kt + chk][:], w2t, 2, 'k'),
            (z_rhs, w3t, 4, 'v'),
        )
        for (rhs_fn, wt_, bo, who) in srcs1:
            for mt in range(2):
                pq = [PS.tile([128, 9, SLAB_C], F32, tag='ps',
                              name=f'p{who}{mt}_{i}') for i in range(2)]
                for kt in range(2):
                    for chk in range(2):
                        nc.tensor.matmul(
                            pq[chk][:],
                            lhsT=wt_[:, 256 * kt + 128 * mt:
                                     256 * kt + 128 * mt + 128],
                            rhs=rhs_fn(kt, chk),
                            start=(kt == 0), stop=(kt == 1))
                bcol = b32[:, bo + mt:bo + mt + 1]
                for chk in range(2):
                    rsel = slice(9 * chk, 9 * chk + 9)
                    if who == 'q':
                        nc.scalar.add(qst[mt][:, rsel, :], pq[chk][:], bcol)
                    elif who == 'k':
                        nc.vector.tensor_scalar_add(kst[mt][:, rsel, :],
                                                    pq[chk][:], bcol)
                        nc.vector.scalar_tensor_tensor(
                            kmst[mt][:, 0:PX].rearrange(
                                'p (r c) -> p r c', c=SLAB_C)[:, rsel, :],
                            pq[chk][:], bcol, pes[:, rsel, :],
                            op0=ALU.add, op1=ALU.subtract)
                    else:
                        nc.scalar.add(vst[mt][:, rsel, :], pq[chk][:], bcol)

        # ---- stage 2: vT (pixel-major v, no bias) from z ----
        for pb in range(9):
            pvt = PS.tile([128, 256], F32, tag='ps', name=f'pvt{pb}')
            for kt in range(2):
                nc.tensor.matmul(
                    pvt[:],
                    lhsT=ztc[kt][:, 108 * pb:108 * pb + 128],
                    rhs=w3t[:, 256 * kt:256 * kt + 256],
                    start=(kt == 0), stop=(kt == 1))
            nc.vector.tensor_scalar_add(
                vtsc[pb][:, :, 0:64],
                pvt[0:108, :].rearrange('p (h d) -> p h d', d=64), 0.0)
            nc.vector.memset(vtsc[pb][:, :, 64], 2.0)

        # ---- attention (chunk-major QK; AV with stationary vts) ----
        att_sb = [att0, att1]
        for t in range(2):
            for hh in range(2):
                h = 2 * t + hh
                hp = 64 * hh
                attE_t = {}
                pav = {}
                for ct in range(9):
                    rb0 = max(0, ct - 3)
                    rb1 = min(5, ct)
                    n = rb1 - rb0 + 1
                    patt = PSQ.tile([128, 384], F32, tag='qk')
                    nc.tensor.matmul(
                        patt[:, 0:96 * n],
                        lhsT=kmst[t][hp:hp + 64, 108 * ct:108 * ct + 128],
                        rhs=qst[t][hp:hp + 64, 3 + 2 * rb0:3 + 2 * rb0 + 2 * n, 3:51],
                        start=True, stop=True)
                    aE = AE.tile([108, 384], F16, tag='attE')
                    attE_t[ct] = aE
                    nc.scalar.activation(aE[:, 0:96 * n], patt[0:108, 0:96 * n],
                                         AF.Exp, scale=0.125)
                    # reversed mask: block b of a full chunk is cc = 3-b;
                    # partial chunks are contiguous slices of it
                    moff = 96 * (3 - ct) if ct < 3 else 0
                    nc.vector.tensor_mul(
                        aE[:, 0:96 * n],
                        aE[:, 0:96 * n],
                        mart[0:108, moff:moff + 96 * n])
                    for rb in range(rb0, rb1 + 1):
                        b = rb - rb0
                        if ct == rb:
                            pav[rb] = PSA.tile([65, 96], F32, tag='av',
                                               name=f'pav{t}_{hh}_{rb}')
                        nc.tensor.matmul(
                            pav[rb][:],
                            lhsT=vtsc[ct][:, h, :],
                            rhs=attE_t[ct][:, 96 * b:96 * b + 96],
                            start=(ct == rb), stop=(ct == rb + 3))
                        if ct == rb + 3:
                            nc.vector.tensor_scalar_add(
                                att_sb[t][0:65, hh, rb, :], pav[rb][:], 0.0)
            nc.sync.dma_start(
                out=oatt_d[t],
                in_=att_sb[t][:].rearrange('p a b c -> p (a b c)'))

        # ---- conv branch f-stage: reads raw q/k/v slab windows; the
        # validity mask is applied at the psum->sbuf copy ----
        srcs = [(qst, 0), (qst, 1), (kst, 0), (kst, 1), (vst, 0), (vst, 1)]
        mcv = mct[:].rearrange('p (r c) -> p r c', c=CGC)
        for ig in range(6):
            gm = GM6[ig]
            for chk in range(2):
                pf = PS.tile([128, 7, CGC], F32, tag='ps', name=f'pf{ig}_{chk}')
                for kti, (srcl, tt_) in enumerate(srcs):
                    nc.tensor.matmul(
                        pf[0:gm, :],
                        lhsT=cwft[:, OFF_F[ig] + kti * gm:OFF_F[ig] + (kti + 1) * gm],
                        rhs=srcl[tt_][:, 2 + 7 * chk:2 + 7 * chk + 7, 2:52],
                        start=(kti == 0), stop=(kti == 5))
                nc.vector.tensor_mul(
                    fsg[ig][0:gm, 7 * chk:7 * chk + 7, :], pf[0:gm, :],
                    mcv[0:gm, 7 * chk:7 * chk + 7, :])

        # ---- depthwise: per (tt,hf,ig) a clean 9-shift accumulation
        # group in its own psum; copy to staging, DMA to channel range ----
        cwd_v = cwdt[:].rearrange('p (s m) -> p s m', m=52)
        dq = [nc.sync, nc.scalar]
        for tt in range(2):
            for hf in range(2):
                for ig in range(3 * tt, 3 * tt + 3):
                    gw = 4 * GSIZE6[ig]
                    gk = GM6[ig]
                    pig = PSA.tile([gw, 6, 48], F32, tag='av',
                                   name=f'pd{tt}_{hf}_{ig}')
                    si = 0
                    for dr in (-1, 0, 1):
                        for dc in (-1, 0, 1):
                            nc.tensor.matmul(
                                pig[:],
                                lhsT=cwd_v[0:gk, ig * 9 + si, 0:gw],
                                rhs=fsg[ig][0:gk, 1 + 6 * hf + dr:7 + 6 * hf + dr,
                                            1 + dc:49 + dc],
                                start=(si == 0), stop=(si == 8))
                            si += 1
                    sg = SG.tile([gw, 6, 48], F16, tag='sg')
                    if ig % 2 == 0:
                        nc.scalar.activation(sg[:], pig[:], AF.Copy)
                    else:
                        nc.vector.tensor_scalar_add(sg[:], pig[:], 0.0)
                    cb = 4 * GSTART6[ig] - 128 * tt
                    dq[ig % 2].dma_start(
                        out=ocv_d[tt, hf, cb:cb + gw, :],
                        in_=sg[:].rearrange('p a b -> p (a b)'))

    nc.finalize()
    return nc


def _host_prep(inputs):
    """Build per-core input maps (packed per-section blobs)."""
    x, y, z = inputs['x'], inputs['y'], inputs['z']
    W1, b1 = inputs['W1'], inputs['b1']
    W2, b2 = inputs['W2'], inputs['b2']
    W3, b3 = inputs['W3'], inputs['b3']
    Wp, bp = inputs['Wp'], inputs['bp']
    Wfc, Wdep = inputs['Wfc'], inputs['Wdep']

    f32, f16 = np.float32, np.float16

    def pad_rc(a):  # reflect-pad H and W by 3: [B, C, 54, 54]
        return np.pad(a, ((0, 0), (0, 0), (PAD, PAD), (PAD, PAD)), mode='reflect')

    xp, yp, zp = pad_rc(x), pad_rc(y), pad_rc(z)

    loc = np.linspace(-1.0, 1.0, W, dtype=f32)
    locp = np.pad(loc, PAD, mode='reflect')        # [54] padded positions
    dd = np.arange(128) % 64
    aX = Wp.astype(f32)[dd, 0][:, None] * locp[None, :]   # [128,54]

    def wpack(Wm):
        wtr = np.ascontiguousarray(Wm.T.astype(f16)).reshape(2, 128, 256)
        return np.ascontiguousarray(wtr.transpose(1, 0, 2).reshape(128, 512))

    w1p, w2p, w3p = wpack(W1), wpack(W2), wpack(W3)

    # reversed attention mask [108, 4, 96]: block b holds cc = 3-b
    ma = np.zeros((2, SLAB_C, 4, 2, W), f16)
    for wr in range(2):
        for cp in range(SLAB_C):
            for cc in range(4):
                for r2 in range(2):
                    if 0 <= 2 * cc + wr - r2 <= 6:
                        for c in range(W):
                            if 0 <= cp - c <= 6:
                                ma[wr, cp, cc, r2, c] = 1.0
    ma_r = np.ascontiguousarray(ma[:, :, ::-1].reshape(108, 384))

    # compact block-diagonal f weights [128, NCWF]
    bdf = np.zeros((128, NCWF), f16)
    for ig in range(6):
        gs, gn, gm = GSTART6[ig], GSIZE6[ig], GM6[ig]
        for kti in range(6):
            tau, tt = kti // 2, kti % 2
            base = OFF_F[ig] + kti * gm
            for hp in range(2):
                cprime = 4 * tau + 2 * tt + hp
                for gl in range(gn):
                    for i in range(9):
                        bdf[64 * hp + gs + gl, base + gl * 9 + i] = Wfc[i, cprime]

    # compact depthwise weights [117, 54, 52] (0.5 folded in)
    bdd = np.zeros((117, 54, 52), f16)
    for ig in range(6):
        gs, gn = GSTART6[ig], GSIZE6[ig]
        for si, (dr, dc) in enumerate([(a, b) for a in (-1, 0, 1) for b in (-1, 0, 1)]):
            kh, kw = dr + 1, dc + 1
            for gl in range(gn):
                for i in range(9):
                    for op in range(4):
                        bdd[gl * 9 + i, ig * 9 + si, 4 * gl + op] = \
                            0.5 * Wdep[4 * (gs + gl) + op, i, kh, kw]
    pcwd = bdd.reshape(117, NCWD)

    # fp32 bias blob
    b32a = np.zeros((128, 6), f32)
    b32a[:, 0:2] = b1.astype(f32).reshape(2, 128).T
    b32a[:, 2:4] = b2.astype(f32).reshape(2, 128).T
    b32a[:, 4:6] = b3.astype(f32).reshape(2, 128).T

    in_maps = []
    for core in range(NCORES):
        bi, blk = core // 4, core % 4
        r0 = RPC * blk

        def slab(ap):
            s = ap[bi][:, r0:r0 + SLAB_R, :].astype(f16).reshape(256, PX)
            out = np.zeros((128, 2 * PXP), f16)
            for t in range(2):
                out[:, PXP * t:PXP * t + PX] = s[128 * t:128 * t + 128]
            return out

        # per-core bY: Wp[d,1]*loc_h_padded[r0+r] + bp[d]
        sm = np.zeros((128, 72), f32)
        sm[:, 0:SLAB_C] = aX
        rows = locp[r0:r0 + SLAB_R].astype(f32)
        sm[:, SLAB_C:SLAB_C + SLAB_R] = (
            Wp.astype(f32)[dd, 1][:, None] * rows[None, :]
            + bp.astype(f32)[dd][:, None])

        mc = np.zeros((CGR, CGC), f16)
        for i in range(CGR):
            if 0 <= r0 - 1 + i < H:
                mc[i, 1:49] = 1.0

        in_maps.append({'w1': w1p, 'w2': w2p, 'w3': w3p, 'b32': b32a,
                        'p_sm': sm, 'p_mar': ma_r, 'p_mc': np.broadcast_to(mc.reshape(1, CPX), (128, CPX)).copy(),
                        'p_x': slab(xp), 'p_y': slab(yp), 'p_z': slab(zp),
                        'p_cwf': bdf, 'p_cwd': pcwd})
    return in_maps


def kernel(**inputs):
    global last_results
    from concourse.bass_utils import run_bass_kernel_spmd

    if 'nc' not in _CACHE:
        _CACHE['nc'] = _build_nc()
    nc = _CACHE['nc']

    in_maps = _host_prep(inputs)
    trace = bool(os.environ.get('BASS_TRACE'))
    res = run_bass_kernel_spmd(nc, in_maps, list(range(NCORES)), trace=trace)
    last_results = res

    b3 = inputs['b3'].astype(np.float32)
    out = np.zeros((B, C, H, W), np.float32)
    for core in range(NCORES):
        bi, blk = core // 4, core % 4
        r0 = RPC * blk
        att = res.results[core]['out_att'].astype(np.float32) \
            .reshape(2, 65, 2, 6, 2, W)
        cnv = res.results[core]['out_conv'].astype(np.float32) \
            .reshape(2, 2, 128, 6, W)
        for t in range(2):
            for hh in range(2):
                h = 2 * t + hh
                a = att[t, 0:64, hh]           # [64, 6, 2, 48]
                den = att[t, 64, hh]           # [6, 2, 48]
                out[bi, 64 * h:64 * h + 64, r0:r0 + RPC, :] = \
                    (a / den[None]).reshape(64, RPC, W) \
                    + 0.5 * b3[64 * h:64 * h + 64, None, None]
        for tt in range(2):
            for hf in range(2):
                out[bi, 128 * tt:128 * tt + 128,
                    r0 + 6 * hf:r0 + 6 * hf + 6, :] += cnv[tt, hf]
    return out


# revision 11
# speedup vs baseline: 1.0416x; 1.0416x over previous
"""Trainium2 Bass kernel for GCFM sparse-attention module.

Sharding: 8 cores = 2 batches x 4 row-blocks (12 rows each).
Each core gets reflect-padded input slabs (18 rows x 54 cols) so the
7x7 local-attention window needs no boundary logic on device; the
conv branch's zero-padding is handled with a 0/1 validity mask.

v4 restructure:
  - fine-grained tiles (per (kt,chk) x/y pieces, per-kt z, per-mt
    q/k/v/kms, per-pb vts, per-ig fs) so dependency tracking never
    over-serializes on whole-tensor granularity.
  - input DMAs split into ~2us pieces spread over the 3 DMA queues
    (sync/scalar/gpsimd) ordered by first-use time.
  - 14 warm-up matmuls on scratch data ramp the PE clock to max
    p-state while the first input DMAs are still in flight.
  - position encoding built on device from a rank-2 blob (aX + bY),
    conv validity mask shipped as one row and partition-broadcast.
  - kms = (k_psum + b2) - pe fused into one DVE op reading PSUM.
  - conv f-stage reads raw q/k/v slabs; validity mask applied at the
    psum->sbuf copy (mask commutes with channel mixing).
  - depthwise weights shipped compact ([117, 54, 52], 0.66MB instead
    of 1.7MB); each (tt,hf,ig) is a clean 9-matmul accumulation group
    in its own psum, copied to a staging tile and DMA'd to its
    channel range in DRAM.
  - vts carries no b3 bias (host adds 0.5*b3 after normalize).
  - att + conv outputs shipped as f16.

Math notes:
  - att logits = (1/8) * q . unfold(k - pe); the q.pe term is constant
    over the window axis and cancels in softmax.
  - AV rhs includes a constant 2.0 column so psum row 64 = 2*sum(exp);
    host computes att/row64 = 0.5 * softmax-weighted v + 0.5*b3.
  - QK runs chunk-major: one matmul per (head, 2-window-row chunk)
    covering all row-blocks that use the chunk (N up to 384).
"""

import os
import numpy as np

# ---- hardcoded problem geometry ----
B, C, H, W = 2, 256, 48, 48
HEAD, D = 4, 64
KATT = 7
PAD = KATT // 2          # 3
NCORES = 8
RPC = H // 4             # 12 rows per core
SLAB_R, SLAB_C = RPC + 2 * PAD, W + 2 * PAD   # 18, 54
PX = SLAB_R * SLAB_C     # 972
PXP = PX + 32            # padded slab stride so 128-wide chunk reads fit
CGR, CGC = RPC + 2, W + 2   # conv grid 14 x 50
CPX = CGR * CGC          # 700
GSIZE6 = [13, 13, 6, 13, 13, 6]
GSTART6 = [0, 13, 26, 32, 45, 58]
GM6 = [9 * g for g in GSIZE6]            # f-block col widths (117/54)
OFF_F = [0]
for _g in GM6:
    OFF_F.append(OFF_F[-1] + 6 * _g)     # compact cwf offsets, total 3456
NCWF = OFF_F[-1]
NCWD = 54 * 52                           # compact cwd cols

_CACHE = {}
last_results = None      # BassKernelResults from the most recent run


def _build_nc():
    """Build the (shared SPMD) Bass program once."""
    import concourse.bacc as bacc
    import concourse.mybir as mybir
    from concourse.tile import TileContext
    from contextlib import ExitStack

    F32 = mybir.dt.float32
    F16 = mybir.dt.float16
    AF = mybir.ActivationFunctionType
    ALU = mybir.AluOpType

    nc = bacc.Bacc(None, target_bir_lowering=False)

    w1_d = nc.declare_dram_parameter('w1', [128, 512], F16, isOutput=False)
    w2_d = nc.declare_dram_parameter('w2', [128, 512], F16, isOutput=False)
    w3_d = nc.declare_dram_parameter('w3', [128, 512], F16, isOutput=False)
    b32_d = nc.declare_dram_parameter('b32', [128, 6], F32, isOutput=False)
    sm_d = nc.declare_dram_parameter('p_sm', [128, 72], F32, isOutput=False)
    mar_d = nc.declare_dram_parameter('p_mar', [108, 384], F16, isOutput=False)
    mc_d = nc.declare_dram_parameter('p_mc', [128, CPX], F16, isOutput=False)
    px_d = nc.declare_dram_parameter('p_x', [128, 2 * PXP], F16, isOutput=False)
    py_d = nc.declare_dram_parameter('p_y', [128, 2 * PXP], F16, isOutput=False)
    pz_d = nc.declare_dram_parameter('p_z', [128, 2 * PXP], F16, isOutput=False)
    pcwf_d = nc.declare_dram_parameter('p_cwf', [128, NCWF], F16, isOutput=False)
    pcwd_d = nc.declare_dram_parameter('p_cwd', [117, NCWD], F16, isOutput=False)
    oatt_d = nc.declare_dram_parameter('out_att', [2, 65, 1152], F16, isOutput=True)
    ocv_d = nc.declare_dram_parameter('out_conv', [2, 2, 128, 288], F16, isOutput=True)

    with TileContext(nc) as tc, ExitStack() as ctx:
        P = ctx.enter_context(tc.tile_pool(name='persist', bufs=1))
        AE = ctx.enter_context(tc.tile_pool(name='attE', bufs=18))
        SG = ctx.enter_context(tc.tile_pool(name='stg', bufs=3))
        PS = ctx.enter_context(tc.tile_pool(name='ps', bufs=2, space='PSUM'))
        PSQ = ctx.enter_context(tc.tile_pool(name='psq', bufs=2, space='PSUM'))
        PSA = ctx.enter_context(tc.tile_pool(name='psa', bufs=4, space='PSUM'))

        # ---- persistent SBUF tensors ----
        w1t = P.tile([128, 512], F16, tag='w1t')
        w2t = P.tile([128, 512], F16, tag='w2t')
        w3t = P.tile([128, 512], F16, tag='w3t')
        b32 = P.tile([128, 6], F32, tag='b32')
        smt = P.tile([128, 72], F32, tag='smt')
        mart = P.tile([108, 384], F16, tag='mart')
        mct = P.tile([128, CPX], F16, tag='mct')
        xts = [P.tile([128, 9, SLAB_C], F16, tag=f'xt{i}', name=f'xts{i}') for i in range(4)]
        yts = [P.tile([128, 9, SLAB_C], F16, tag=f'yt{i}', name=f'yts{i}') for i in range(4)]
        ztc = [P.tile([128, PXP], F16, tag=f'zt{i}', name=f'ztc{i}') for i in range(2)]
        pes = P.tile([128, SLAB_R, SLAB_C], F16, tag='pes')
        cwft = P.tile([128, NCWF], F16, tag='cwft')
        cwdt = P.tile([117, NCWD], F16, tag='cwdt')

        qst = [P.tile([128, SLAB_R, SLAB_C], F16, tag=f'qs{i}', name=f'qst{i}') for i in range(2)]
        kst = [P.tile([128, SLAB_R, SLAB_C], F16, tag=f'ks{i}', name=f'kst{i}') for i in range(2)]
        vst = [P.tile([128, SLAB_R, SLAB_C], F16, tag=f'vs{i}', name=f'vst{i}') for i in range(2)]
        kmst = [P.tile([128, PXP], F16, tag=f'kms{i}', name=f'kmst{i}') for i in range(2)]
        vtsc = [P.tile([108, 4, 65], F16, tag=f'vts{i}', name=f'vtsc{i}') for i in range(9)]
        att0 = P.tile([65, 2, 6, 96], F16, tag='att0')
        att1 = P.tile([65, 2, 6, 96], F16, tag='att1')
        fsg = [P.tile([128, CGR, CGC], F16, tag=f'fs{i}', name=f'fsg{i}') for i in range(6)]
        wu = P.tile([128, 512], F16, tag='wu')

        # ---- input DMAs: 3 queues, ~2us pieces, ordered by first use ----
        def xy_in(dram, kt, chk):
            return dram[:, PXP * kt + 486 * chk:PXP * kt + 486 * chk + 486] \
                .rearrange('p (r c) -> p r c', c=SLAB_C)

        CWF3 = [0, 1404, 2430, NCWF]     # ig01 | ig23 | ig45
        nc.sync.dma_start(out=w1t[:], in_=w1_d[:])
        nc.sync.dma_start(out=xts[0][:], in_=xy_in(px_d, 0, 0))
        nc.sync.dma_start(out=w2t[:], in_=w2_d[:])
        nc.sync.dma_start(out=ztc[0][:, 0:486], in_=pz_d[:, 0:486])
        nc.sync.dma_start(out=w3t[:], in_=w3_d[:])
        nc.sync.dma_start(out=cwft[:, CWF3[0]:CWF3[1]], in_=pcwf_d[:, CWF3[0]:CWF3[1]])
        nc.sync.dma_start(out=cwdt[:, 0:936], in_=pcwd_d[:, 0:936])

        nc.scalar.dma_start(out=b32[:], in_=b32_d[:])
        nc.scalar.dma_start(out=smt[:], in_=sm_d[:])
        nc.scalar.dma_start(out=xts[2][:], in_=xy_in(px_d, 1, 0))
        nc.scalar.dma_start(out=yts[0][:], in_=xy_in(py_d, 0, 0))
        nc.scalar.dma_start(out=yts[2][:], in_=xy_in(py_d, 1, 0))
        nc.scalar.dma_start(out=ztc[0][:, 486:PXP], in_=pz_d[:, 486:PXP])
        nc.scalar.dma_start(out=mart[:], in_=mar_d[:])
        nc.scalar.dma_start(out=cwft[:, CWF3[1]:CWF3[2]], in_=pcwf_d[:, CWF3[1]:CWF3[2]])
        nc.scalar.dma_start(out=cwdt[:, 936:1872], in_=pcwd_d[:, 936:1872])

        nc.gpsimd.dma_start(out=xts[1][:], in_=xy_in(px_d, 0, 1))
        nc.gpsimd.dma_start(out=xts[3][:], in_=xy_in(px_d, 1, 1))
        nc.gpsimd.dma_start(out=yts[1][:], in_=xy_in(py_d, 0, 1))
        nc.gpsimd.dma_start(out=yts[3][:], in_=xy_in(py_d, 1, 1))
        nc.gpsimd.dma_start(out=ztc[1][:, 0:486], in_=pz_d[:, PXP:PXP + 486])
        nc.gpsimd.dma_start(out=ztc[1][:, 486:PXP], in_=pz_d[:, PXP + 486:2 * PXP])
        nc.gpsimd.dma_start(out=mct[:], in_=mc_d[:])
        nc.gpsimd.dma_start(out=cwft[:, CWF3[2]:CWF3[3]], in_=pcwf_d[:, CWF3[2]:CWF3[3]])
        nc.gpsimd.dma_start(out=cwdt[:, 1872:NCWD], in_=pcwd_d[:, 1872:NCWD])

        # ---- PE warm-up: ramp the clock while input DMAs fly ----
        nc.vector.memset(wu[:], 0.001)
        pwu = PS.tile([128, 512], F32, tag='ps', name='warm')
        for i in range(14):
            nc.tensor.matmul(pwu[:], lhsT=wu[:, 0:128], rhs=wu[:],
                             start=(i == 0), stop=(i == 13))
        nc.vector.tensor_scalar_add(wu[:, 0:2], pwu[:, 0:2], 0.0)

        # ---- small device-side constructions ----
        # pe[p, r, c] = aX[p, c] + bY[p, r]  (rank-2 position encoding)
        for r in range(SLAB_R):
            nc.vector.tensor_scalar_add(pes[:, r, :], smt[:, 0:SLAB_C],
                                        smt[:, SLAB_C + r:SLAB_C + r + 1])
        for mt in range(2):
            nc.vector.memset(kmst[mt][:, PX:PXP], 0.0)

        # ---- stage 1: 1x1 convs q,k,v ----
        def z_rhs(kt, chk):
            return ztc[kt][:, 0:PX].rearrange('p (r c) -> p r c', c=SLAB_C)[
                :, 9 * chk:9 * chk + 9, :]

        srcs1 = (
            (lambda kt, chk: xts[2 * kt + chk][:], w1t, 0, 'q'),
            (lambda kt, chk: yts[2 * kt + chk][:], w2t, 2, 'k'),
            (z_rhs, w3t, 4, 'v'),
        )
        for (rhs_fn, wt_, bo, who) in srcs1:
            for mt in range(2):
                pq = [PS.tile([128, 9, SLAB_C], F32, tag='ps',
                              name=f'p{who}{mt}_{i}') for i in range(2)]
                for kt in range(2):
                    for chk in range(2):
                        nc.tensor.matmul(
                            pq[chk][:],
                            lhsT=wt_[:, 256 * kt + 128 * mt:
                                     256 * kt + 128 * mt + 128],
                            rhs=rhs_fn(kt, chk),
                            start=(kt == 0), stop=(kt == 1))
                bcol = b32[:, bo + mt:bo + mt + 1]
                for chk in range(2):
                    rsel = slice(9 * chk, 9 * chk + 9)
                    if who == 'q':
                        nc.scalar.add(qst[mt][:, rsel, :], pq[chk][:], bcol)
                    elif who == 'k':
                        nc.vector.tensor_scalar_add(kst[mt][:, rsel, :],
                                                    pq[chk][:], bcol)
                        nc.vector.scalar_tensor_tensor(
                            kmst[mt][:, 0:PX].rearrange(
                                'p (r c) -> p r c', c=SLAB_C)[:, rsel, :],
                            pq[chk][:], bcol, pes[:, rsel, :],
                            op0=ALU.add, op1=ALU.subtract)
                    else:
                        nc.scalar.add(vst[mt][:, rsel, :], pq[chk][:], bcol)

        # ---- stage 2: vT (pixel-major v, no bias) from z ----
        for pb in range(9):
            pvt = PS.tile([128, 256], F32, tag='ps', name=f'pvt{pb}')
            for kt in range(2):
                nc.tensor.matmul(
                    pvt[:],
                    lhsT=ztc[kt][:, 108 * pb:108 * pb + 128],
                    rhs=w3t[:, 256 * kt:256 * kt + 256],
                    start=(kt == 0), stop=(kt == 1))
            nc.vector.tensor_scalar_add(
                vtsc[pb][:, :, 0:64],
                pvt[0:108, :].rearrange('p (h d) -> p h d', d=64), 0.0)
            nc.vector.memset(vtsc[pb][:, :, 64], 2.0)

        # ---- attention (chunk-major QK; AV with stationary vts) ----
        att_sb = [att0, att1]
        for t in range(2):
            for hh in range(2):
                h = 2 * t + hh
                hp = 64 * hh
                attE_t = {}
                pav = {}
                for ct in range(9):
                    rb0 = max(0, ct - 3)
                    rb1 = min(5, ct)
                    n = rb1 - rb0 + 1
                    patt = PSQ.tile([128, 384], F32, tag='qk')
                    nc.tensor.matmul(
                        patt[:, 0:96 * n],
                        lhsT=kmst[t][hp:hp + 64, 108 * ct:108 * ct + 128],
                        rhs=qst[t][hp:hp + 64, 3 + 2 * rb0:3 + 2 * rb0 + 2 * n, 3:51],
                        start=True, stop=True)
                    aE = AE.tile([108, 384], F16, tag='attE')
                    attE_t[ct] = aE
                    nc.scalar.activation(aE[:, 0:96 * n], patt[0:108, 0:96 * n],
                                         AF.Exp, scale=0.125)
                    # reversed mask: block b of a full chunk is cc = 3-b;
                    # partial chunks are contiguous slices of it
                    moff = 96 * (3 - ct) if ct < 3 else 0
                    nc.vector.tensor_mul(
                        aE[:, 0:96 * n],
                        aE[:, 0:96 * n],
                        mart[0:108, moff:moff + 96 * n])
                    for rb in range(rb0, rb1 + 1):
                        b = rb - rb0
                        if ct == rb:
                            pav[rb] = PSA.tile([65, 96], F32, tag='av',
                                               name=f'pav{t}_{hh}_{rb}')
                        nc.tensor.matmul(
                            pav[rb][:],
                            lhsT=vtsc[ct][:, h, :],
                            rhs=attE_t[ct][:, 96 * b:96 * b + 96],
                            start=(ct == rb), stop=(ct == rb + 3))
                        if ct == rb + 3:
                            nc.vector.tensor_scalar_add(
                                att_sb[t][0:65, hh, rb, :], pav[rb][:], 0.0)
            nc.sync.dma_start(
                out=oatt_d[t],
                in_=att_sb[t][:].rearrange('p a b c -> p (a b c)'))

        # ---- conv branch f-stage: reads raw q/k/v slab windows; the
        # validity mask is applied at the psum->sbuf copy ----
        srcs = [(qst, 0), (qst, 1), (kst, 0), (kst, 1), (vst, 0), (vst, 1)]
        mcv = mct[:].rearrange('p (r c) -> p r c', c=CGC)
        for ig in range(6):
            gm = GM6[ig]
            for chk in range(2):
                pf = PS.tile([128, 7, CGC], F32, tag='ps', name=f'pf{ig}_{chk}')
                for kti, (srcl, tt_) in enumerate(srcs):
                    nc.tensor.matmul(
                        pf[0:gm, :],
                        lhsT=cwft[:, OFF_F[ig] + kti * gm:OFF_F[ig] + (kti + 1) * gm],
                        rhs=srcl[tt_][:, 2 + 7 * chk:2 + 7 * chk + 7, 2:52],
                        start=(kti == 0), stop=(kti == 5))
                if chk == 0:
                    nc.vector.tensor_mul(fsg[ig][0:gm, 0:7, :], pf[0:gm, :],
                                         mcv[0:gm, 0:7, :])
                else:
                    nc.scalar.copy(fsg[ig][0:gm, 7:14, :], pf[0:gm, :])
                    nc.gpsimd.tensor_mul(fsg[ig][0:gm, 7:14, :],
                                         fsg[ig][0:gm, 7:14, :],
                                         mcv[0:gm, 7:14, :])

        # ---- depthwise: per (tt,hf,ig) a clean 9-shift accumulation
        # group in its own psum; copy to staging, DMA to channel range ----
        cwd_v = cwdt[:].rearrange('p (s m) -> p s m', m=52)
        dq = [nc.sync, nc.scalar]
        for tt in range(2):
            for hf in range(2):
                for ig in range(3 * tt, 3 * tt + 3):
                    gw = 4 * GSIZE6[ig]
                    gk = GM6[ig]
                    pig = PSA.tile([gw, 6, 48], F32, tag='av',
                                   name=f'pd{tt}_{hf}_{ig}')
                    si = 0
                    for dr in (-1, 0, 1):
                        for dc in (-1, 0, 1):
                            nc.tensor.matmul(
                                pig[:],
                                lhsT=cwd_v[0:gk, ig * 9 + si, 0:gw],
                                rhs=fsg[ig][0:gk, 1 + 6 * hf + dr:7 + 6 * hf + dr,
                                            1 + dc:49 + dc],
                                start=(si == 0), stop=(si == 8))
                            si += 1
                    sg = SG.tile([gw, 6, 48], F16, tag='sg')
                    if ig % 2 == 0:
                        nc.scalar.activation(sg[:], pig[:], AF.Copy)
                    else:
                        nc.vector.tensor_scalar_add(sg[:], pig[:], 0.0)
                    cb = 4 * GSTART6[ig] - 128 * tt
                    dq[ig % 2].dma_start(
                        out=ocv_d[tt, hf, cb:cb + gw, :],
                        in_=sg[:].rearrange('p a b -> p (a b)'))

    nc.finalize()
    return nc


def _host_prep(inputs):
    """Build per-core input maps (packed per-section blobs)."""
    x, y, z = inputs['x'], inputs['y'], inputs['z']
    W1, b1 = inputs['W1'], inputs['b1']
    W2, b2 = inputs['W2'], inputs['b2']
    W3, b3 = inputs['W3'], inputs['b3']
    Wp, bp = inputs['Wp'], inputs['bp']
    Wfc, Wdep = inputs['Wfc'], inputs['Wdep']

    f32, f16 = np.float32, np.float16

    def pad_rc(a):  # reflect-pad H and W by 3: [B, C, 54, 54]
        return np.pad(a, ((0, 0), (0, 0), (PAD, PAD), (PAD, PAD)), mode='reflect')

    xp, yp, zp = pad_rc(x), pad_rc(y), pad_rc(z)

    loc = np.linspace(-1.0, 1.0, W, dtype=f32)
    locp = np.pad(loc, PAD, mode='reflect')        # [54] padded positions
    dd = np.arange(128) % 64
    aX = Wp.astype(f32)[dd, 0][:, None] * locp[None, :]   # [128,54]

    def wpack(Wm):
        wtr = np.ascontiguousarray(Wm.T.astype(f16)).reshape(2, 128, 256)
        return np.ascontiguousarray(wtr.transpose(1, 0, 2).reshape(128, 512))

    w1p, w2p, w3p = wpack(W1), wpack(W2), wpack(W3)

    # reversed attention mask [108, 4, 96]: block b holds cc = 3-b
    ma = np.zeros((2, SLAB_C, 4, 2, W), f16)
    for wr in range(2):
        for cp in range(SLAB_C):
            for cc in range(4):
                for r2 in range(2):
                    if 0 <= 2 * cc + wr - r2 <= 6:
                        for c in range(W):
                            if 0 <= cp - c <= 6:
                                ma[wr, cp, cc, r2, c] = 1.0
    ma_r = np.ascontiguousarray(ma[:, :, ::-1].reshape(108, 384))

    # compact block-diagonal f weights [128, NCWF]
    bdf = np.zeros((128, NCWF), f16)
    for ig in range(6):
        gs, gn, gm = GSTART6[ig], GSIZE6[ig], GM6[ig]
        for kti in range(6):
            tau, tt = kti // 2, kti % 2
            base = OFF_F[ig] + kti * gm
            for hp in range(2):
                cprime = 4 * tau + 2 * tt + hp
                for gl in range(gn):
                    for i in range(9):
                        bdf[64 * hp + gs + gl, base + gl * 9 + i] = Wfc[i, cprime]

    # compact depthwise weights [117, 54, 52] (0.5 folded in)
    bdd = np.zeros((117, 54, 52), f16)
    for ig in range(6):
        gs, gn = GSTART6[ig], GSIZE6[ig]
        for si, (dr, dc) in enumerate([(a, b) for a in (-1, 0, 1) for b in (-1, 0, 1)]):
            kh, kw = dr + 1, dc + 1
            for gl in range(gn):
                for i in range(9):
                    for op in range(4):
                        bdd[gl * 9 + i, ig * 9 + si, 4 * gl + op] = \
                            0.5 * Wdep[4 * (gs + gl) + op, i, kh, kw]
    pcwd = bdd.reshape(117, NCWD)

    # fp32 bias blob
    b32a = np.zeros((128, 6), f32)
    b32a[:, 0:2] = b1.astype(f32).reshape(2, 128).T
    b32a[:, 2:4] = b2.astype(f32).reshape(2, 128).T
    b32a[:, 4:6] = b3.astype(f32).reshape(2, 128).T

    in_maps = []
    for core in range(NCORES):
        bi, blk = core // 4, core % 4
        r0 = RPC * blk

        def slab(ap):
            s = ap[bi][:, r0:r0 + SLAB_R, :].astype(f16).reshape(256, PX)
            out = np.zeros((128, 2 * PXP), f16)
            for t in range(2):
                out[:, PXP * t:PXP * t + PX] = s[128 * t:128 * t + 128]
            return out

        # per-core bY: Wp[d,1]*loc_h_padded[r0+r] + bp[d]
        sm = np.zeros((128, 72), f32)
        sm[:, 0:SLAB_C] = aX
        rows = locp[r0:r0 + SLAB_R].astype(f32)
        sm[:, SLAB_C:SLAB_C + SLAB_R] = (
            Wp.astype(f32)[dd, 1][:, None] * rows[None, :]
            + bp.astype(f32)[dd][:, None])

        mc = np.zeros((CGR, CGC), f16)
        for i in range(CGR):
            if 0 <= r0 - 1 + i < H:
                mc[i, 1:49] = 1.0

        in_maps.append({'w1': w1p, 'w2': w2p, 'w3': w3p, 'b32': b32a,
                        'p_sm': sm, 'p_mar': ma_r, 'p_mc': np.broadcast_to(mc.reshape(1, CPX), (128, CPX)).copy(),
                        'p_x': slab(xp), 'p_y': slab(yp), 'p_z': slab(zp),
                        'p_cwf': bdf, 'p_cwd': pcwd})
    return in_maps


def kernel(**inputs):
    global last_results
    from concourse.bass_utils import run_bass_kernel_spmd

    if 'nc' not in _CACHE:
        _CACHE['nc'] = _build_nc()
    nc = _CACHE['nc']

    in_maps = _host_prep(inputs)
    trace = bool(os.environ.get('BASS_TRACE'))
    res = run_bass_kernel_spmd(nc, in_maps, list(range(NCORES)), trace=trace)
    last_results = res

    b3 = inputs['b3'].astype(np.float32)
    out = np.zeros((B, C, H, W), np.float32)
    for core in range(NCORES):
        bi, blk = core // 4, core % 4
        r0 = RPC * blk
        att = res.results[core]['out_att'].astype(np.float32) \
            .reshape(2, 65, 2, 6, 2, W)
        cnv = res.results[core]['out_conv'].astype(np.float32) \
            .reshape(2, 2, 128, 6, W)
        for t in range(2):
            for hh in range(2):
                h = 2 * t + hh
                a = att[t, 0:64, hh]           # [64, 6, 2, 48]
                den = att[t, 64, hh]           # [6, 2, 48]
                out[bi, 64 * h:64 * h + 64, r0:r0 + RPC, :] = \
                    (a / den[None]).reshape(64, RPC, W) \
                    + 0.5 * b3[64 * h:64 * h + 64, None, None]
        for tt in range(2):
            for hf in range(2):
                out[bi, 128 * tt:128 * tt + 128,
                    r0 + 6 * hf:r0 + 6 * hf + 6, :] += cnv[tt, hf]
    return out
